# revision 5
# baseline (speedup 1.0000x reference)
"""Low_Rank_linear Trainium2 kernel (v3: batched DMA, A0-A1-B0-B1 phase order).

Math (reference):
    hidden = (x[..., col_idx] * wnorm) @ B.T            # [tok, 512]
    y[..., row_idx]      = hidden @ A.T + x[..., col_comp_idx] @ sparse1.T
    y[..., row_comp_idx] = x @ sparse2.T

All index handling is done on the host (free): x's columns are permuted by
perm = [col_idx; col_comp_idx] and transposed to feature-major xt [4096, tok];
the output is computed in permuted column order and scattered back on host.

Device math per core (1024 tokens), all bf16 operands / fp32 PSUM:
    hidden.T [512, tok]  = (B*wnorm).T.T @ xt[:3840]       (K=3840)
    ycomp.T  [256, tok]  = s2p.T.T @ xt                    (K=4096)
    y_rows   [tok, 3840] = [hidden | x_comp] @ [A | s1].T  (K=768)
where x_comp rows of the lhsT come directly from xt k-tiles 30..31.

Schedule: MM-A for both token blocks first (keeps the w2 load off the
critical path), then MM-B for both blocks. Inputs stream on the two HWDGE
queues (sync: w1/s2 then w2; scalar: xt), outputs go out via gpsimd.
PSUM->SBUF copies alternate between vector and scalar engines.
"""

import numpy as np
import ml_dtypes

import concourse.bacc as bacc
import concourse.tile as tile
import concourse.mybir as mybir
from concourse.bass_utils import run_bass_kernel_spmd

N_CORES = 8
TOK = 8192            # 4 * 2048 tokens total
TPC = TOK // N_CORES  # 1024 tokens per core
N = 4096              # model width (in == out)
RANK = 512
NKEEP = 3840          # kept columns/rows
NCOMP = 256           # complement set size (both col and row)
KV = RANK + NCOMP     # 768 = contraction width of second matmul
BLK = 512             # token block (matmul moving N)
KT = N // 128         # 32 x-feature k-tiles
KT1 = NKEEP // 128    # 30 k-tiles for the hidden matmul
KT2 = KV // 128       # 6 k-tiles for the second matmul
NB = TPC // BLK       # 2 token blocks per core

_BF16 = mybir.dt.bfloat16
_F32 = mybir.dt.float32


def _chunks():
    """k-tile chunk list: small chunks first so compute starts immediately,
    4-kt chunks in the middle, 2-kt chunks near the tail so late k-tiles
    arrive with fine granularity."""
    sizes = (1, 1, 2) + (2,) * 14
    out, pos = [], 0
    for sz in sizes:
        out.append((pos, sz))
        pos += sz
    assert pos == KT
    return out


def _build_nc():
    nc = bacc.Bacc(None)
    xt_d = nc.dram_tensor("xt", [N, TPC], _BF16, kind="ExternalInput")
    w1_d = nc.dram_tensor("w1t", [NKEEP, RANK], _BF16, kind="ExternalInput")
    s2_d = nc.dram_tensor("s2t", [N, NCOMP], _BF16, kind="ExternalInput")
    w2_d = nc.dram_tensor("w2t", [KV, NKEEP], _BF16, kind="ExternalInput")
    yr_d = nc.dram_tensor("yr", [TPC, NKEEP], _BF16, kind="ExternalOutput")
    yc_d = nc.dram_tensor("yc", [NCOMP, TPC], _BF16, kind="ExternalOutput")

    xt_r = xt_d.rearrange("(kt p) t -> p kt t", p=128)   # [128, 32, 1024]
    w1_r = w1_d.rearrange("(kt p) m -> p kt m", p=128)   # [128, 30, 512]
    s2_r = s2_d.rearrange("(kt p) m -> p kt m", p=128)   # [128, 32, 256]
    w2_r = w2_d.rearrange("(kt p) n -> p kt n", p=128)   # [128, 6, 3840]

    with tile.TileContext(nc) as tc:
        with (
            tc.tile_pool(name="wts", bufs=1) as w_pool,
            tc.tile_pool(name="xin", bufs=1) as x_pool,
            tc.tile_pool(name="hid", bufs=1) as h_pool,
            tc.tile_pool(name="yo", bufs=6) as yo_pool,
            tc.tile_pool(name="yct", bufs=2) as yc_pool,
            tc.tile_pool(name="psA", bufs=4, space="PSUM") as psA,
            tc.tile_pool(name="psC", bufs=2, space="PSUM") as psC,
            tc.tile_pool(name="psB", bufs=2, space="PSUM") as psB,
        ):
            w1_sb = w_pool.tile([128, KT1, RANK], _BF16)
            s2_sb = w_pool.tile([128, KT, NCOMP], _BF16)
            w2_sb = w_pool.tile([128, KT2, NKEEP], _BF16)
            xt_sb = x_pool.tile([128, KT, TPC], _BF16)
            hT_sb = h_pool.tile([128, KT2 - 2, TPC], _BF16)

            # A couple of dummy matmuls so PE activity (and the HAM
            # clock-gate warmup window) starts at the tensor preamble end
            # instead of first-data arrival. Only enough to fill the
            # pre-data window: cold *real* matmuls still do half-rate work,
            # dummies past that point do none. Operand tile is zeroed by
            # the otherwise-idle gpsimd engine; the result is never read.
            warm_sb = w_pool.tile([128, BLK], _BF16)
            nc.gpsimd.memset(warm_sb[:], 0.0)
            warm_ps = psB.tile([128, BLK], _F32, name="warm_ps", tag="psb")
            for i in range(2):
                nc.tensor.matmul(
                    warm_ps[:], warm_sb[:, 0:128], warm_sb[:],
                    start=True, stop=True,
                )

            # Inputs stream on the two HWDGE queues in strict need order:
            # per k-tile group, one queue takes w1+s2, the other takes
            # block-0's half of xt, alternating. MM-A0 then only demands
            # ~250 GB/s, comfortably under the per-core HBM limit, so the
            # tensor engine never waits on the stream. Block-1's xt follows
            # (needed by MM-A1 from ~52us), then w2 (MM-B, ~90us).
            for i, (k0, sz) in enumerate(_chunks()):
                wq, xq = (nc.sync, nc.scalar) if i % 2 == 0 else (
                    nc.scalar, nc.sync)
                if k0 < KT1:
                    s1z = min(sz, KT1 - k0)
                    wq.dma_start(
                        w1_sb[:, k0 : k0 + s1z, :], w1_r[:, k0 : k0 + s1z, :]
                    )
                wq.dma_start(
                    s2_sb[:, k0 : k0 + sz, :], s2_r[:, k0 : k0 + sz, :]
                )
                xq.dma_start(
                    xt_sb[:, k0 : k0 + sz, 0:BLK],
                    xt_r[:, k0 : k0 + sz, 0:BLK],
                )
            for i in range(4):
                q = nc.sync if i % 2 == 0 else nc.scalar
                q.dma_start(
                    xt_sb[:, i * 8 : (i + 1) * 8, BLK:TPC],
                    xt_r[:, i * 8 : (i + 1) * 8, BLK:TPC],
                )
            nc.sync.dma_start(w2_sb[:, 0:3, :], w2_r[:, 0:3, :])
            nc.scalar.dma_start(w2_sb[:, 3:6, :], w2_r[:, 3:6, :])

            copy_engines = [nc.vector.tensor_copy, nc.scalar.copy]
            dma_engines = [nc.sync, nc.scalar]

            # ---- MM-A for both blocks ----
            for b in range(NB):
                tok = slice(b * BLK, (b + 1) * BLK)
                psa = [
                    psA.tile([128, BLK], _F32, name=f"psa{b}_{m}", tag="psa")
                    for m in range(4)
                ]
                psc = [
                    psC.tile([128, BLK], _F32, name=f"psc{b}_{m}", tag="psc")
                    for m in range(2)
                ]
                for kt in range(KT):
                    rhs = xt_sb[:, kt, tok]
                    if kt < KT1:
                        for m in range(4):
                            nc.tensor.matmul(
                                psa[m][:],
                                w1_sb[:, kt, m * 128 : (m + 1) * 128],
                                rhs,
                                start=(kt == 0),
                                stop=(kt == KT1 - 1),
                            )
                    if kt < KT - 2:
                        for m in range(2):
                            nc.tensor.matmul(
                                psc[m][:],
                                s2_sb[:, kt, m * 128 : (m + 1) * 128],
                                rhs,
                                start=(kt == 0),
                                stop=False,
                            )
                # hidden -> bf16 SBUF split across vector/scalar; tensor
                # meanwhile finishes the two deferred ycomp k-tiles
                for m in range(4):
                    copy_engines[m % 2](out=hT_sb[:, m, tok], in_=psa[m][:])
                for kt in range(KT - 2, KT):
                    rhs = xt_sb[:, kt, tok]
                    for m in range(2):
                        nc.tensor.matmul(
                            psc[m][:],
                            s2_sb[:, kt, m * 128 : (m + 1) * 128],
                            rhs,
                            start=False,
                            stop=(kt == KT - 1),
                        )
                for m in range(2):
                    yct = yc_pool.tile([128, BLK], _BF16, name=f"yct{b}_{m}",
                                       tag="yct")
                    copy_engines[m % 2](out=yct[:], in_=psc[m][:])
                    dma_engines[m % 2].dma_start(
                        yc_d[m * 128 : (m + 1) * 128, tok], yct[:]
                    )

            # ---- MM-B for both blocks ----
            ncopy = 0
            for b in range(NB):
                for t in range(4):
                    toff = b * BLK + t * 128
                    for n in range(8):
                        nn = 512 if n < 7 else 256
                        ps = psB.tile([128, BLK], _F32, name="psb", tag="psb")
                        for kt in range(KT2):
                            if kt < 4:
                                lhs = hT_sb[:, kt, toff : toff + 128]
                            else:
                                lhs = xt_sb[:, KT - 6 + kt, toff : toff + 128]
                            nc.tensor.matmul(
                                ps[:, 0:nn],
                                lhs,
                                w2_sb[:, kt, n * 512 : n * 512 + nn],
                                start=(kt == 0),
                                stop=(kt == KT2 - 1),
                            )
                        yo = yo_pool.tile([128, BLK], _BF16, name="yo",
                                          tag="yo")
                        copy_engines[ncopy % 2](out=yo[:, 0:nn],
                                                in_=ps[:, 0:nn])
                        dma_engines[ncopy % 2].dma_start(
                            yr_d[toff : toff + 128, n * 512 : n * 512 + nn],
                            yo[:, 0:nn],
                        )
                        ncopy += 1
    nc.finalize()
    return nc


_NC_CACHE = {}


def get_nc():
    if "nc" not in _NC_CACHE:
        _NC_CACHE["nc"] = _build_nc()
    return _NC_CACHE["nc"]


def kernel(x, A, B, sparse_weights1, sparse_weights2, weights_norms_rowwise,
           col_idx, col_comp_idx, row_idx, row_comp_idx):
    bf16 = ml_dtypes.bfloat16
    x = np.asarray(x, dtype=np.float32)
    A = np.asarray(A, np.float32)
    B = np.asarray(B, np.float32)
    s1 = np.asarray(sparse_weights1, np.float32)
    s2 = np.asarray(sparse_weights2, np.float32)
    wnorm = np.asarray(weights_norms_rowwise, np.float32)
    col_idx = np.asarray(col_idx).astype(np.int64)
    col_comp_idx = np.asarray(col_comp_idx).astype(np.int64)
    row_idx = np.asarray(row_idx).astype(np.int64)
    row_comp_idx = np.asarray(row_comp_idx).astype(np.int64)

    perm = np.concatenate([col_idx, col_comp_idx])
    w1t = np.ascontiguousarray((B * wnorm[None, :]).T).astype(bf16)
    s2t = np.ascontiguousarray(s2.T[perm]).astype(bf16)
    w2t = np.ascontiguousarray(np.concatenate([A, s1], axis=1).T).astype(bf16)

    xbf = x.reshape(TOK, N).astype(bf16)
    xt_full = np.ascontiguousarray(xbf.T)[perm]          # [4096, 8192]

    nc = get_nc()
    in_maps = [
        {
            "xt": np.ascontiguousarray(xt_full[:, c * TPC : (c + 1) * TPC]),
            "w1t": w1t,
            "s2t": s2t,
            "w2t": w2t,
        }
        for c in range(N_CORES)
    ]
    res = run_bass_kernel_spmd(nc, in_maps, core_ids=list(range(N_CORES)))
    globals()["_LAST_RESULTS"] = res
    yr = np.concatenate(
        [res.results[c]["yr"] for c in range(N_CORES)], axis=0
    ).astype(np.float32)
    yc = np.concatenate(
        [res.results[c]["yc"].T for c in range(N_CORES)], axis=0
    ).astype(np.float32)
    y = np.empty((TOK, N), np.float32)
    y[:, row_idx] = yr
    y[:, row_comp_idx] = yc
    return y.reshape(x.shape)


# revision 6
# speedup vs baseline: 1.0811x; 1.0811x over previous
"""Low_Rank_linear Trainium2 kernel (dense minimal-FLOP formulation).

Math (reference):
    hidden = (x[..., col_idx] * wnorm) @ B.T            # [tok, 512]
    y[..., row_idx]      = hidden @ A.T + x[..., col_comp_idx] @ sparse1.T
    y[..., row_comp_idx] = x @ sparse2.T

All index handling is done on the host (free): x's columns are permuted by
perm = [col_idx; col_comp_idx] and transposed to feature-major xt [4096, tok];
the output is computed in permuted column order and scattered back on host.

Device math per core (1024 tokens), all bf16 operands / fp32 PSUM:
    hidden.T [512, tok]  = (B*wnorm).T.T @ xt[:3840]       (K=3840)
    ycomp.T  [256, tok]  = s2p.T.T @ xt                    (K=4096)
    y_rows   [tok, 3840] = [hidden | x_comp] @ [A | s1].T  (K=768)
where x_comp rows of the lhsT come directly from xt k-tiles 30..31.

Schedule: MM-A for both token blocks first (keeps the w2 load off the
critical path), then MM-B for both blocks. Inputs stream on the two HWDGE
queues (sync: w1/s2 then w2; scalar: xt), outputs go out via gpsimd.
PSUM->SBUF copies alternate between vector and scalar engines.
"""

import numpy as np
import ml_dtypes

import concourse.bacc as bacc
import concourse.tile as tile
import concourse.mybir as mybir
from concourse.bass_utils import run_bass_kernel_spmd

N_CORES = 8
TOK = 8192            # 4 * 2048 tokens total
TPC = TOK // N_CORES  # 1024 tokens per core
N = 4096              # model width (in == out)
RANK = 512
NKEEP = 3840          # kept columns/rows
NCOMP = 256           # complement set size (both col and row)
KV = RANK + NCOMP     # 768 = contraction width of second matmul
BLK = 512             # token block (matmul moving N)
KT = N // 128         # 32 x-feature k-tiles
KT1 = NKEEP // 128    # 30 k-tiles for the hidden matmul
KT2 = KV // 128       # 6 k-tiles for the second matmul
NB = TPC // BLK       # 2 token blocks per core

_BF16 = mybir.dt.bfloat16
_F32 = mybir.dt.float32


def _chunks():
    """k-tile chunk list: small chunks first so compute starts immediately,
    4-kt chunks in the middle, 2-kt chunks near the tail so late k-tiles
    arrive with fine granularity."""
    sizes = (1, 1, 2) + (2,) * 14
    out, pos = [], 0
    for sz in sizes:
        out.append((pos, sz))
        pos += sz
    assert pos == KT
    return out


def _build_nc():
    nc = bacc.Bacc(None)
    xt_d = nc.dram_tensor("xt", [N, TPC], _BF16, kind="ExternalInput")
    w1_d = nc.dram_tensor("w1t", [NKEEP, RANK], _BF16, kind="ExternalInput")
    s2_d = nc.dram_tensor("s2t", [N, NCOMP], _BF16, kind="ExternalInput")
    w2_d = nc.dram_tensor("w2t", [KV, NKEEP], _BF16, kind="ExternalInput")
    yr_d = nc.dram_tensor("yr", [TPC, NKEEP], _BF16, kind="ExternalOutput")
    yc_d = nc.dram_tensor("yc", [NCOMP, TPC], _BF16, kind="ExternalOutput")

    xt_r = xt_d.rearrange("(kt p) t -> p kt t", p=128)   # [128, 32, 1024]
    w1_r = w1_d.rearrange("(kt p) m -> p kt m", p=128)   # [128, 30, 512]
    s2_r = s2_d.rearrange("(kt p) m -> p kt m", p=128)   # [128, 32, 256]
    w2_r = w2_d.rearrange("(kt p) n -> p kt n", p=128)   # [128, 6, 3840]

    with tile.TileContext(nc) as tc:
        with (
            tc.tile_pool(name="wts", bufs=1) as w_pool,
            tc.tile_pool(name="xin", bufs=1) as x_pool,
            tc.tile_pool(name="hid", bufs=1) as h_pool,
            tc.tile_pool(name="yo", bufs=6) as yo_pool,
            tc.tile_pool(name="yct", bufs=2) as yc_pool,
            tc.tile_pool(name="psA", bufs=4, space="PSUM") as psA,
            tc.tile_pool(name="psC", bufs=2, space="PSUM") as psC,
            tc.tile_pool(name="psB", bufs=2, space="PSUM") as psB,
        ):
            w1_sb = w_pool.tile([128, KT1, RANK], _BF16)
            s2_sb = w_pool.tile([128, KT, NCOMP], _BF16)
            w2_sb = w_pool.tile([128, KT2, NKEEP], _BF16)
            xt_sb = x_pool.tile([128, KT, TPC], _BF16)
            hT_sb = h_pool.tile([128, KT2 - 2, TPC], _BF16)

            # ~4us of dummy matmuls to trip the PE HAM clock gate to full
            # rate while the first input DMAs land. The operand tile is
            # zeroed by the otherwise-idle gpsimd engine (fast MEMSET right
            # after its preamble); the result is never read.
            warm_sb = w_pool.tile([128, BLK], _BF16)
            nc.gpsimd.memset(warm_sb[:], 0.0)
            warm_ps = psB.tile([128, BLK], _F32, name="warm_ps", tag="psb")
            for i in range(10):
                nc.tensor.matmul(
                    warm_ps[:], warm_sb[:, 0:128], warm_sb[:],
                    start=True, stop=True,
                )

            # Inputs stream on the two HWDGE queues in strict need order:
            # per k-tile group, one queue takes w1+s2, the other takes
            # block-0's half of xt, alternating. MM-A0 then only demands
            # ~250 GB/s, comfortably under the per-core HBM limit, so the
            # tensor engine never waits on the stream. Block-1's xt follows
            # (needed by MM-A1 from ~52us), then w2 (MM-B, ~90us).
            for i, (k0, sz) in enumerate(_chunks()):
                wq, xq = (nc.sync, nc.scalar) if i % 2 == 0 else (
                    nc.scalar, nc.sync)
                if k0 < KT1:
                    s1z = min(sz, KT1 - k0)
                    wq.dma_start(
                        w1_sb[:, k0 : k0 + s1z, :], w1_r[:, k0 : k0 + s1z, :]
                    )
                wq.dma_start(
                    s2_sb[:, k0 : k0 + sz, :], s2_r[:, k0 : k0 + sz, :]
                )
                xq.dma_start(
                    xt_sb[:, k0 : k0 + sz, 0:BLK],
                    xt_r[:, k0 : k0 + sz, 0:BLK],
                )
            for i in range(4):
                q = nc.sync if i % 2 == 0 else nc.scalar
                q.dma_start(
                    xt_sb[:, i * 8 : (i + 1) * 8, BLK:TPC],
                    xt_r[:, i * 8 : (i + 1) * 8, BLK:TPC],
                )
            nc.sync.dma_start(w2_sb[:, 0:3, :], w2_r[:, 0:3, :])
            nc.scalar.dma_start(w2_sb[:, 3:6, :], w2_r[:, 3:6, :])

            copy_engines = [nc.vector.tensor_copy, nc.scalar.copy]
            dma_engines = [nc.sync, nc.scalar]

            # ---- MM-A for both blocks ----
            for b in range(NB):
                tok = slice(b * BLK, (b + 1) * BLK)
                psa = [
                    psA.tile([128, BLK], _F32, name=f"psa{b}_{m}", tag="psa")
                    for m in range(4)
                ]
                psc = [
                    psC.tile([128, BLK], _F32, name=f"psc{b}_{m}", tag="psc")
                    for m in range(2)
                ]
                for kt in range(KT):
                    rhs = xt_sb[:, kt, tok]
                    if kt < KT1:
                        for m in range(4):
                            nc.tensor.matmul(
                                psa[m][:],
                                w1_sb[:, kt, m * 128 : (m + 1) * 128],
                                rhs,
                                start=(kt == 0),
                                stop=(kt == KT1 - 1),
                            )
                    if kt < KT - 2:
                        for m in range(2):
                            nc.tensor.matmul(
                                psc[m][:],
                                s2_sb[:, kt, m * 128 : (m + 1) * 128],
                                rhs,
                                start=(kt == 0),
                                stop=False,
                            )
                # hidden -> bf16 SBUF split across vector/scalar; tensor
                # meanwhile finishes the two deferred ycomp k-tiles
                for m in range(4):
                    copy_engines[m % 2](out=hT_sb[:, m, tok], in_=psa[m][:])
                for kt in range(KT - 2, KT):
                    rhs = xt_sb[:, kt, tok]
                    for m in range(2):
                        nc.tensor.matmul(
                            psc[m][:],
                            s2_sb[:, kt, m * 128 : (m + 1) * 128],
                            rhs,
                            start=False,
                            stop=(kt == KT - 1),
                        )
                for m in range(2):
                    yct = yc_pool.tile([128, BLK], _BF16, name=f"yct{b}_{m}",
                                       tag="yct")
                    copy_engines[m % 2](out=yct[:], in_=psc[m][:])
                    dma_engines[m % 2].dma_start(
                        yc_d[m * 128 : (m + 1) * 128, tok], yct[:]
                    )

            # ---- MM-B for both blocks ----
            ncopy = 0
            for b in range(NB):
                for t in range(4):
                    toff = b * BLK + t * 128
                    for n in range(8):
                        nn = 512 if n < 7 else 256
                        ps = psB.tile([128, BLK], _F32, name="psb", tag="psb")
                        for kt in range(KT2):
                            if kt < 4:
                                lhs = hT_sb[:, kt, toff : toff + 128]
                            else:
                                lhs = xt_sb[:, KT - 6 + kt, toff : toff + 128]
                            nc.tensor.matmul(
                                ps[:, 0:nn],
                                lhs,
                                w2_sb[:, kt, n * 512 : n * 512 + nn],
                                start=(kt == 0),
                                stop=(kt == KT2 - 1),
                            )
                        yo = yo_pool.tile([128, BLK], _BF16, name="yo",
                                          tag="yo")
                        copy_engines[ncopy % 2](out=yo[:, 0:nn],
                                                in_=ps[:, 0:nn])
                        dma_engines[ncopy % 2].dma_start(
                            yr_d[toff : toff + 128, n * 512 : n * 512 + nn],
                            yo[:, 0:nn],
                        )
                        ncopy += 1
    nc.finalize()
    return nc


_NC_CACHE = {}


def get_nc():
    if "nc" not in _NC_CACHE:
        _NC_CACHE["nc"] = _build_nc()
    return _NC_CACHE["nc"]


def kernel(x, A, B, sparse_weights1, sparse_weights2, weights_norms_rowwise,
           col_idx, col_comp_idx, row_idx, row_comp_idx):
    bf16 = ml_dtypes.bfloat16
    x = np.asarray(x, dtype=np.float32)
    A = np.asarray(A, np.float32)
    B = np.asarray(B, np.float32)
    s1 = np.asarray(sparse_weights1, np.float32)
    s2 = np.asarray(sparse_weights2, np.float32)
    wnorm = np.asarray(weights_norms_rowwise, np.float32)
    col_idx = np.asarray(col_idx).astype(np.int64)
    col_comp_idx = np.asarray(col_comp_idx).astype(np.int64)
    row_idx = np.asarray(row_idx).astype(np.int64)
    row_comp_idx = np.asarray(row_comp_idx).astype(np.int64)

    perm = np.concatenate([col_idx, col_comp_idx])
    w1t = np.ascontiguousarray((B * wnorm[None, :]).T).astype(bf16)
    s2t = np.ascontiguousarray(s2.T[perm]).astype(bf16)
    w2t = np.ascontiguousarray(np.concatenate([A, s1], axis=1).T).astype(bf16)

    xbf = x.reshape(TOK, N).astype(bf16)
    xt_full = np.ascontiguousarray(xbf.T)[perm]          # [4096, 8192]

    nc = get_nc()
    in_maps = [
        {
            "xt": np.ascontiguousarray(xt_full[:, c * TPC : (c + 1) * TPC]),
            "w1t": w1t,
            "s2t": s2t,
            "w2t": w2t,
        }
        for c in range(N_CORES)
    ]
    res = run_bass_kernel_spmd(nc, in_maps, core_ids=list(range(N_CORES)))
    globals()["_LAST_RESULTS"] = res
    yr = np.concatenate(
        [res.results[c]["yr"] for c in range(N_CORES)], axis=0
    ).astype(np.float32)
    yc = np.concatenate(
        [res.results[c]["yc"].T for c in range(N_CORES)], axis=0
    ).astype(np.float32)
    y = np.empty((TOK, N), np.float32)
    y[:, row_idx] = yr
    y[:, row_comp_idx] = yc
    return y.reshape(x.shape)


# revision 7
# speedup vs baseline: 1.0860x; 1.0045x over previous
"""Low_Rank_linear Trainium2 kernel (dense minimal-FLOP formulation).

Math (reference):
    hidden = (x[..., col_idx] * wnorm) @ B.T            # [tok, 512]
    y[..., row_idx]      = hidden @ A.T + x[..., col_comp_idx] @ sparse1.T
    y[..., row_comp_idx] = x @ sparse2.T

All index handling is done on the host (free): x's columns are permuted by
perm = [col_idx; col_comp_idx] and transposed to feature-major xt [4096, tok];
the output is computed in permuted column order and scattered back on host.

Device math per core (1024 tokens), all bf16 operands / fp32 PSUM:
    hidden.T [512, tok]  = (B*wnorm).T.T @ xt[:3840]       (K=3840)
    ycomp.T  [256, tok]  = s2p.T.T @ xt                    (K=4096)
    y_rows   [tok, 3840] = [hidden | x_comp] @ [A | s1].T  (K=768)
where x_comp rows of the lhsT come directly from xt k-tiles 30..31.

Schedule: MM-A for both token blocks first (keeps the w2 load off the
critical path), then MM-B for both blocks. Inputs stream on the two HWDGE
queues (sync: w1/s2 then w2; scalar: xt), outputs go out via gpsimd.
PSUM->SBUF copies alternate between vector and scalar engines.
"""

import numpy as np
import ml_dtypes

import concourse.bacc as bacc
import concourse.tile as tile
import concourse.mybir as mybir
from concourse.bass_utils import run_bass_kernel_spmd

N_CORES = 8
TOK = 8192            # 4 * 2048 tokens total
TPC = TOK // N_CORES  # 1024 tokens per core
N = 4096              # model width (in == out)
RANK = 512
NKEEP = 3840          # kept columns/rows
NCOMP = 256           # complement set size (both col and row)
KV = RANK + NCOMP     # 768 = contraction width of second matmul
BLK = 512             # token block (matmul moving N)
KT = N // 128         # 32 x-feature k-tiles
KT1 = NKEEP // 128    # 30 k-tiles for the hidden matmul
KT2 = KV // 128       # 6 k-tiles for the second matmul
NB = TPC // BLK       # 2 token blocks per core

_BF16 = mybir.dt.bfloat16
_F32 = mybir.dt.float32


def _chunks():
    """k-tile chunk list: small chunks first so compute starts immediately,
    4-kt chunks in the middle, 2-kt chunks near the tail so late k-tiles
    arrive with fine granularity."""
    sizes = (1, 1, 2) + (2,) * 14
    out, pos = [], 0
    for sz in sizes:
        out.append((pos, sz))
        pos += sz
    assert pos == KT
    return out


def _build_nc():
    nc = bacc.Bacc(None)
    xt_d = nc.dram_tensor("xt", [N, TPC], _BF16, kind="ExternalInput")
    w1_d = nc.dram_tensor("w1t", [NKEEP, RANK], _BF16, kind="ExternalInput")
    s2_d = nc.dram_tensor("s2t", [N, NCOMP], _BF16, kind="ExternalInput")
    w2_d = nc.dram_tensor("w2t", [KV, NKEEP], _BF16, kind="ExternalInput")
    yr_d = nc.dram_tensor("yr", [TPC, NKEEP], _BF16, kind="ExternalOutput")
    yc_d = nc.dram_tensor("yc", [NCOMP, TPC], _BF16, kind="ExternalOutput")

    xt_r = xt_d.rearrange("(kt p) t -> p kt t", p=128)   # [128, 32, 1024]
    w1_r = w1_d.rearrange("(kt p) m -> p kt m", p=128)   # [128, 30, 512]
    s2_r = s2_d.rearrange("(kt p) m -> p kt m", p=128)   # [128, 32, 256]
    w2_r = w2_d.rearrange("(kt p) n -> p kt n", p=128)   # [128, 6, 3840]

    with tile.TileContext(nc) as tc:
        with (
            tc.tile_pool(name="wts", bufs=1) as w_pool,
            tc.tile_pool(name="xin", bufs=1) as x_pool,
            tc.tile_pool(name="hid", bufs=1) as h_pool,
            tc.tile_pool(name="yo", bufs=6) as yo_pool,
            tc.tile_pool(name="yct", bufs=2) as yc_pool,
            tc.tile_pool(name="psA", bufs=4, space="PSUM") as psA,
            tc.tile_pool(name="psC", bufs=2, space="PSUM") as psC,
            tc.tile_pool(name="psB", bufs=2, space="PSUM") as psB,
        ):
            w1_sb = w_pool.tile([128, KT1, RANK], _BF16)
            s2_sb = w_pool.tile([128, KT, NCOMP], _BF16)
            w2_sb = w_pool.tile([128, KT2, NKEEP], _BF16)
            xt_sb = x_pool.tile([128, KT, TPC], _BF16)
            hT_sb = h_pool.tile([128, KT2 - 2, TPC], _BF16)

            # A couple of dummy matmuls so PE activity (and the HAM
            # clock-gate warmup window) starts at the tensor preamble end
            # instead of first-data arrival. Only enough to fill the
            # pre-data window: cold *real* matmuls still do half-rate work,
            # dummies past that point do none. Operand tile is zeroed by
            # the otherwise-idle gpsimd engine; the result is never read.
            warm_sb = w_pool.tile([128, BLK], _BF16)
            nc.gpsimd.memset(warm_sb[:], 0.0)
            warm_ps = psB.tile([128, BLK], _F32, name="warm_ps", tag="psb")
            for i in range(2):
                nc.tensor.matmul(
                    warm_ps[:], warm_sb[:, 0:128], warm_sb[:],
                    start=True, stop=True,
                )

            # Inputs stream on the two HWDGE queues in strict need order:
            # per k-tile group, one queue takes w1+s2, the other takes
            # block-0's half of xt, alternating. MM-A0 then only demands
            # ~250 GB/s, comfortably under the per-core HBM limit, so the
            # tensor engine never waits on the stream. Block-1's xt follows
            # (needed by MM-A1 from ~52us), then w2 (MM-B, ~90us).
            for i, (k0, sz) in enumerate(_chunks()):
                wq, xq = (nc.sync, nc.scalar) if i % 2 == 0 else (
                    nc.scalar, nc.sync)
                if k0 < KT1:
                    s1z = min(sz, KT1 - k0)
                    wq.dma_start(
                        w1_sb[:, k0 : k0 + s1z, :], w1_r[:, k0 : k0 + s1z, :]
                    )
                wq.dma_start(
                    s2_sb[:, k0 : k0 + sz, :], s2_r[:, k0 : k0 + sz, :]
                )
                xq.dma_start(
                    xt_sb[:, k0 : k0 + sz, 0:BLK],
                    xt_r[:, k0 : k0 + sz, 0:BLK],
                )
            for i in range(4):
                q = nc.sync if i % 2 == 0 else nc.scalar
                q.dma_start(
                    xt_sb[:, i * 8 : (i + 1) * 8, BLK:TPC],
                    xt_r[:, i * 8 : (i + 1) * 8, BLK:TPC],
                )
            nc.sync.dma_start(w2_sb[:, 0:3, :], w2_r[:, 0:3, :])
            nc.scalar.dma_start(w2_sb[:, 3:6, :], w2_r[:, 3:6, :])

            copy_engines = [nc.vector.tensor_copy, nc.scalar.copy]
            dma_engines = [nc.sync, nc.scalar]

            # ---- MM-A for both blocks ----
            for b in range(NB):
                tok = slice(b * BLK, (b + 1) * BLK)
                psa = [
                    psA.tile([128, BLK], _F32, name=f"psa{b}_{m}", tag="psa")
                    for m in range(4)
                ]
                psc = [
                    psC.tile([128, BLK], _F32, name=f"psc{b}_{m}", tag="psc")
                    for m in range(2)
                ]
                for kt in range(KT):
                    rhs = xt_sb[:, kt, tok]
                    if kt < KT1:
                        for m in range(4):
                            nc.tensor.matmul(
                                psa[m][:],
                                w1_sb[:, kt, m * 128 : (m + 1) * 128],
                                rhs,
                                start=(kt == 0),
                                stop=(kt == KT1 - 1),
                            )
                    if kt < KT - 2:
                        for m in range(2):
                            nc.tensor.matmul(
                                psc[m][:],
                                s2_sb[:, kt, m * 128 : (m + 1) * 128],
                                rhs,
                                start=(kt == 0),
                                stop=False,
                            )
                # hidden -> bf16 SBUF split across vector/scalar; tensor
                # meanwhile finishes the two deferred ycomp k-tiles
                for m in range(4):
                    copy_engines[m % 2](out=hT_sb[:, m, tok], in_=psa[m][:])
                for kt in range(KT - 2, KT):
                    rhs = xt_sb[:, kt, tok]
                    for m in range(2):
                        nc.tensor.matmul(
                            psc[m][:],
                            s2_sb[:, kt, m * 128 : (m + 1) * 128],
                            rhs,
                            start=False,
                            stop=(kt == KT - 1),
                        )
                for m in range(2):
                    yct = yc_pool.tile([128, BLK], _BF16, name=f"yct{b}_{m}",
                                       tag="yct")
                    copy_engines[m % 2](out=yct[:], in_=psc[m][:])
                    dma_engines[m % 2].dma_start(
                        yc_d[m * 128 : (m + 1) * 128, tok], yct[:]
                    )

            # ---- MM-B for both blocks ----
            ncopy = 0
            for b in range(NB):
                for t in range(4):
                    toff = b * BLK + t * 128
                    for n in range(8):
                        nn = 512 if n < 7 else 256
                        ps = psB.tile([128, BLK], _F32, name="psb", tag="psb")
                        for kt in range(KT2):
                            if kt < 4:
                                lhs = hT_sb[:, kt, toff : toff + 128]
                            else:
                                lhs = xt_sb[:, KT - 6 + kt, toff : toff + 128]
                            nc.tensor.matmul(
                                ps[:, 0:nn],
                                lhs,
                                w2_sb[:, kt, n * 512 : n * 512 + nn],
                                start=(kt == 0),
                                stop=(kt == KT2 - 1),
                            )
                        yo = yo_pool.tile([128, BLK], _BF16, name="yo",
                                          tag="yo")
                        copy_engines[ncopy % 2](out=yo[:, 0:nn],
                                                in_=ps[:, 0:nn])
                        dma_engines[ncopy % 2].dma_start(
                            yr_d[toff : toff + 128, n * 512 : n * 512 + nn],
                            yo[:, 0:nn],
                        )
                        ncopy += 1
    nc.finalize()
    return nc


_NC_CACHE = {}


def get_nc():
    if "nc" not in _NC_CACHE:
        _NC_CACHE["nc"] = _build_nc()
    return _NC_CACHE["nc"]


def kernel(x, A, B, sparse_weights1, sparse_weights2, weights_norms_rowwise,
           col_idx, col_comp_idx, row_idx, row_comp_idx):
    bf16 = ml_dtypes.bfloat16
    x = np.asarray(x, dtype=np.float32)
    A = np.asarray(A, np.float32)
    B = np.asarray(B, np.float32)
    s1 = np.asarray(sparse_weights1, np.float32)
    s2 = np.asarray(sparse_weights2, np.float32)
    wnorm = np.asarray(weights_norms_rowwise, np.float32)
    col_idx = np.asarray(col_idx).astype(np.int64)
    col_comp_idx = np.asarray(col_comp_idx).astype(np.int64)
    row_idx = np.asarray(row_idx).astype(np.int64)
    row_comp_idx = np.asarray(row_comp_idx).astype(np.int64)

    perm = np.concatenate([col_idx, col_comp_idx])
    w1t = np.ascontiguousarray((B * wnorm[None, :]).T).astype(bf16)
    s2t = np.ascontiguousarray(s2.T[perm]).astype(bf16)
    w2t = np.ascontiguousarray(np.concatenate([A, s1], axis=1).T).astype(bf16)

    xbf = x.reshape(TOK, N).astype(bf16)
    xt_full = np.ascontiguousarray(xbf.T)[perm]          # [4096, 8192]

    nc = get_nc()
    in_maps = [
        {
            "xt": np.ascontiguousarray(xt_full[:, c * TPC : (c + 1) * TPC]),
            "w1t": w1t,
            "s2t": s2t,
            "w2t": w2t,
        }
        for c in range(N_CORES)
    ]
    res = run_bass_kernel_spmd(nc, in_maps, core_ids=list(range(N_CORES)))
    globals()["_LAST_RESULTS"] = res
    yr = np.concatenate(
        [res.results[c]["yr"] for c in range(N_CORES)], axis=0
    ).astype(np.float32)
    yc = np.concatenate(
        [res.results[c]["yc"].T for c in range(N_CORES)], axis=0
    ).astype(np.float32)
    y = np.empty((TOK, N), np.float32)
    y[:, row_idx] = yr
    y[:, row_comp_idx] = yc
    return y.reshape(x.shape)
